# revision 1
# baseline (speedup 1.0000x reference)
"""PointPillar loss on 8 Trainium2 NeuronCores.

Data-parallel over the batch dim (B=8 -> one batch element per core).
Each core gathers the ~1150 elements of loc/clf that the loss actually
touches (one dma_gather of 256B rows + an on-chip one-hot select),
computes its partial smooth-L1 / focal sums on-device, and the host sums
the 8 partial scalars.

Self-contained: hardcodes the problem shapes from the spec.
"""

import sys

import numpy as np

if "/opt/trn_rl_repo" not in sys.path:
    sys.path.insert(0, "/opt/trn_rl_repo")

B, A, H, W = 8, 2, 496, 432
N_BOXES, N_BG = 50, 1000
PLANE = H * W  # 214272
N_CORES = 8
COLS = 9
N_SLOTS = 128 * COLS  # 1152 slots; 1150 used
CHUNK = 64            # dma_gather row size in f32 elements (256B)
N_ROWS = 4 * PLANE // CHUNK  # 13392
ALPHA = 0.25
BETA_LOC = 2.0

# smalls[128, 132] column layout (f32 view)
IDX0, IDX1 = 0, 36     # dma_gather row indices, int16 bits ([128, 72] i16)
REM0, REM1 = 36, 45    # element position within gathered row
G0, G1 = 45, 47        # gt-box coordinate pairs
INVDA = 47             # 1/sqrt(anchor_w^2 + anchor_h^2)
WF0, WF1 = 48, 57      # focal weights (0 on smooth-L1/pad slots)
WS0, WS1 = 57, 66      # smooth-L1 weights (0 elsewhere)
C0, C1 = 66, 68        # coefficients turning gt pairs into x_gt / y_gt
IO0, IO1 = 68, 132     # iota 0..63
SMALL_COLS = 132

_CACHE = {}


def _grid(flat):
    """Map a length-1152 slot vector to the on-chip [128, 9] layout.

    Slot n lives at partition n % 128, free column n // 128 (dma_gather's
    native output order) — so slots 0..99 (the smooth-L1 entries) occupy
    column 0, one per partition, letting the gt target act as a
    per-partition scalar operand.
    """
    return np.ascontiguousarray(flat.reshape(COLS, 128).T)


def _const_cols():
    wf = np.zeros(N_SLOTS, np.float32)
    wf[100:150] = -ALPHA / ((B - 1) * (N_BOXES - 1))
    wf[150:1150] = -ALPHA / ((B - 1) * (N_BG - 1))
    ws = np.zeros(N_SLOTS, np.float32)
    ws[0:100] = 0.5 * BETA_LOC / (B * N_BOXES)
    c = np.zeros((128, 2), np.float32)
    c[0:50] = (0.5, 0.5)    # x_gt = 0.5*c0 + 0.5*c2
    c[50:100] = (1.5, -0.5)  # y_gt = 1.5*c1 - 0.5*c3
    return _grid(wf), _grid(ws), c


_WF2D, _WS2D, _C2D = _const_cols()


def build_bass(skip_par=False, skip_act=False, no_dve_sems=False,
               no_gather=False, no_in=False, no_out=False):
    import concourse.bacc as bacc
    import concourse.bass as bass
    import concourse.mybir as mybir
    from concourse import bass_isa
    from concourse.library_config import mlp
    from contextlib import ExitStack

    f32 = mybir.dt.float32
    i16 = mybir.dt.int16
    op = mybir.AluOpType
    act = mybir.ActivationFunctionType

    nc = bacc.Bacc("TRN2", target_bir_lowering=False, debug=False,
                   num_devices=N_CORES)
    planes = nc.dram_tensor("planes", [N_ROWS, CHUNK], f32, kind="ExternalInput")
    smalls = nc.dram_tensor("smalls", [128, SMALL_COLS], f32, kind="ExternalInput")
    outp = nc.dram_tensor("out", [1, 1], f32, kind="ExternalOutput")

    with ExitStack() as ctx:
        block = ctx.enter_context(nc.Block())

        def sb(name, shape, dt=f32):
            return ctx.enter_context(nc.sbuf_tensor(name, shape, dt))

        sm = sb("sm", [128, SMALL_COLS])
        v64 = sb("v64", [128, COLS, CHUNK])
        mask3 = sb("mask3", [128, COLS, CHUNK])
        vm = sb("vm", [128, COLS, CHUNK])
        v = sb("v", [128, COLS])
        tg = sb("tg", [128, 2])
        junk2 = sb("junk2", [128, 2])
        t = sb("t", [128, COLS])
        neg = sb("neg", [128, COLS])
        ab = sb("ab", [128, COLS])
        mm1 = sb("mm1", [128, COLS])
        q = sb("q", [128, COLS])
        r = sb("r", [128, COLS])
        s = sb("s", [128, COLS])
        pcl = sb("pcl", [128, COLS])
        lnb = sb("lnb", [128, COLS])
        cb = sb("cb", [128, COLS])
        c2b = sb("c2b", [128, COLS])
        fo = sb("fo", [128, COLS])
        j9a = sb("j9a", [128, COLS])
        j9b = sb("j9b", [128, COLS])
        tot = sb("tot", [128, COLS])
        acc2 = sb("acc2", [128, 1])
        pr = sb("pr", [128, 1])
        warm = sb("warm", [1, 1])
        io = ctx.enter_context(nc.semaphore("io"))
        gs = ctx.enter_context(nc.semaphore("gs"))
        dve_p = ctx.enter_context(nc.semaphore("dve_p"))
        act_done = ctx.enter_context(nc.semaphore("act_done"))
        dve_done = ctx.enter_context(nc.semaphore("dve_done"))
        par_done = ctx.enter_context(nc.semaphore("par_done"))
        od = ctx.enter_context(nc.semaphore("od"))
        dve_c = ctx.enter_context(nc.semaphore("dve_c"))
        act_c = ctx.enter_context(nc.semaphore("act_c"))

        ks = {}

        @block.vector
        def _(d: bass.BassVectorEngine):
            # Every DVE op incs dve_c at completion; dependent ops wait for
            # their producers' counts. Same-engine program order alone does
            # NOT make writes visible on this HW (Tile does the same).
            cnt = [0]

            def step(ins):
                ins.then_inc(dve_c, 1)
                cnt[0] += 1
                return cnt[0]

            def need(k):
                if not no_dve_sems:
                    d.wait_ge(dve_c, k)

            ks.clear()
            d.wait_ge(io, 32)
            # Tg = sum_j G[:, j] * C[:, j]  (per-partition gt target)
            step(d.tensor_tensor(
                out=junk2[:], in0=sm[:, G0:G1], in1=sm[:, C0:C1], op=op.mult
            ))
            need(cnt[0])
            step(d.tensor_reduce(
                out=tg[:, 0:1], in_=junk2[:], axis=mybir.AxisListType.X, op=op.add
            ))
            # one-hot mask: mask3[p, i, j] = (iota[j] == rem[p, i])
            for i in range(COLS):
                step(d.tensor_scalar(
                    out=mask3[:, i, :], in0=sm[:, IO0:IO1],
                    scalar1=sm[:, REM0 + i:REM0 + i + 1], scalar2=None,
                    op0=op.is_equal,
                ))
            d.wait_ge(gs, 16)
            need(cnt[0])  # all masks written
            # select: v[:, i] = sum_j v64[:, i, j] * mask3[:, i, j], one
            # fused multiply-accumulate per column, no deps between them
            for i in range(COLS):
                step(d.scalar_tensor_tensor(
                    out=vm[:, i, :], in0=v64[:, i, :], scalar=1.0,
                    in1=mask3[:, i, :], op0=op.mult, op1=op.mult,
                    accum_out=v[:, i:i + 1],
                ))
            ks["v"] = cnt[0]
            need(cnt[0])  # v ready
            # ln input first so ACT starts ASAP (inc goes to dve_p, not dve_c)
            pcl_ins = d.tensor_scalar(
                out=pcl[:], in0=v[:], scalar1=1e-12, scalar2=None, op0=op.max
            )
            if skip_act:
                step(pcl_ins)
            else:
                pcl_ins.then_inc(dve_p, 1)
            if skip_act:
                # debug path: focal pieces stay on DVE
                cb_k = step(d.tensor_scalar(
                    out=cb[:], in0=v[:], scalar1=-1.0, scalar2=1.0,
                    op0=op.mult, op1=op.add,
                ))
            # t = (v - Tg) / da   (tg/inv settled long ago)
            t_k = step(d.tensor_scalar(
                out=t[:], in0=v[:], scalar1=tg[:, 0:1],
                scalar2=sm[:, INVDA:INVDA + 1], op0=op.subtract, op1=op.mult,
            ))
            if skip_act:
                need(cb_k)
                ks["c2b"] = step(d.tensor_tensor(out=c2b[:], in0=cb[:],
                                                 in1=cb[:], op=op.mult))
            need(t_k)
            # huber*2 = t^2 - (max(|t|,1) - 1)^2;  |t| = max(-t, t) fused
            ab_k = step(d.scalar_tensor_tensor(
                out=ab[:], in0=t[:], scalar=-1.0, in1=t[:],
                op0=op.mult, op1=op.max,
            ))
            step(d.tensor_tensor(out=q[:], in0=t[:], in1=t[:], op=op.mult))
            need(ab_k)
            mm1_k = step(d.tensor_scalar(
                out=mm1[:], in0=ab[:], scalar1=1.0, scalar2=-1.0,
                op0=op.max, op1=op.add,
            ))
            need(mm1_k)
            r_k = step(d.tensor_tensor(out=r[:], in0=mm1[:], in1=mm1[:],
                                       op=op.mult))
            need(r_k)  # q completed earlier; cumulative count covers it
            s_k = step(d.tensor_tensor(out=s[:], in0=q[:], in1=r[:],
                                       op=op.subtract))
            need(s_k)
            j9a_k = step(d.tensor_tensor(out=j9a[:], in0=s[:],
                                         in1=sm[:, WS0:WS1], op=op.mult))
            if not skip_act:
                d.wait_ge(act_done, 1)  # lnb AND (ACT-made) cb/c2b visible
            else:
                need(ks["c2b"])
            fo_k = step(d.tensor_tensor(
                out=fo[:], in0=c2b[:], in1=pcl[:] if skip_act else lnb[:],
                op=op.mult,
            ))
            need(fo_k)
            j9b_k = step(d.tensor_tensor(out=j9b[:], in0=fo[:],
                                         in1=sm[:, WF0:WF1], op=op.mult))
            need(j9b_k)  # covers j9a too
            # tot = j9a + j9b with fused per-partition accumulate
            d.scalar_tensor_tensor(
                out=tot[:], in0=j9a[:], scalar=1.0, in1=j9b[:],
                op0=op.mult, op1=op.add, accum_out=acc2[:],
            ).then_inc(dve_done, 1)

        @block.gpsimd
        def _(g: bass.BassGpSimd):
            g.load_library(mlp)
            nreg = g.to_reg(N_SLOTS)
            g.wait_ge(io, 16)
            # single_packet=False: 1152 idxs -> 73 descriptors per lane, far
            # beyond the 64-descriptor/16KB single-packet limit.
            if no_gather:
                g.sem_inc(gs, 16)
            else:
                g.dma_gather(
                    v64[:], planes[:], sm[:, IDX0:IDX1].bitcast(i16),
                    N_SLOTS, nreg, CHUNK, single_packet=False,
                ).then_inc(gs, 16)
            g.wait_ge(dve_done, 1)
            if skip_par:
                g.memcpy(pr[0:1, 0:1], acc2[0:1, 0:1]).then_inc(par_done, 1)
            else:
                g.partition_all_reduce(
                    pr[:], acc2[:], channels=128,
                    reduce_op=bass_isa.ReduceOp.add,
                ).then_inc(par_done, 1)

        @block.sync
        def _(sync: bass.BassEngine):
            if no_in:
                sync.sem_inc(io, 32)
            else:
                # idx columns first: the gather only needs these (io >= 16);
                # HWDGE completes in FIFO order, io >= 32 implies all of sm.
                sync.dma_start(out=sm[:, IDX0:IDX1], in_=smalls[:, IDX0:IDX1]
                               ).then_inc(io, 16)
                sync.dma_start(out=sm[:, IDX1:], in_=smalls[:, IDX1:]
                               ).then_inc(io, 16)
            sync.wait_ge(par_done, 1)
            if not no_out:
                sync.dma_start(out=outp[:], in_=pr[0:1, 0:1]).then_inc(od, 16)
                sync.wait_ge(od, 16)

        if not skip_act:
            @block.scalar
            def _(sc: bass.BassScalarEngine):
                # warm the Ln table immediately (const input, no DMA dep);
                # Copy/Square co-reside in the natural_log set: no reloads
                sc.activation(warm[:], nc.const_aps.tensor(1.0, (1, 1)),
                              act.Ln)
                sc.wait_ge(dve_c, ks["v"])
                sc.activation(cb[:], v[:], act.Copy, bias=1.0, scale=-1.0
                              ).then_inc(act_c, 1)
                sc.wait_ge(act_c, 1)
                sc.activation(c2b[:], cb[:], act.Square)
                sc.wait_ge(dve_p, 1)
                sc.activation(lnb[:], pcl[:], act.Ln).then_inc(act_done, 1)

    nc.compile()
    return nc


def host_inputs(regression_targets, classification_targets, gt_boxes, loc, clf,
                anchor):
    reg = np.asarray(regression_targets).astype(np.int64)
    cls_t = np.asarray(classification_targets).astype(np.int64)
    gt = np.asarray(gt_boxes, dtype=np.float32)
    loc = np.asarray(loc, dtype=np.float32)
    clf = np.asarray(clf, dtype=np.float32)
    anc = np.asarray(anchor, dtype=np.float32)
    inv_da = np.float32(1.0) / np.sqrt(anc[0] * anc[0] + anc[1] * anc[1],
                                       dtype=np.float32)

    iota = np.arange(CHUNK, dtype=np.float32)

    in_maps = []
    for b in range(B):
        planes_b = np.ascontiguousarray(
            np.stack([loc[b, 0, 0], loc[b, 0, 1], clf[b, 0, 1], clf[b, 0, 0]])
        ).reshape(N_ROWS, CHUNK)
        y, x = reg[b, :, 1], reg[b, :, 0]
        base = y * W + x
        flat = np.zeros(N_SLOTS, np.int64)
        flat[0:50] = 0 * PLANE + base
        flat[50:100] = 1 * PLANE + base
        flat[100:150] = 2 * PLANE + base
        flat[150:1150] = 3 * PLANE + cls_t[b, :, 2] * W + cls_t[b, :, 1]

        # dma_gather index layout: index n sits at partition n % 16,
        # column n // 16, replicated across the 8 groups of 16 partitions.
        rows16 = np.ascontiguousarray(
            (flat // CHUNK).astype(np.int16).reshape(N_SLOTS // 16, 16).T
        )
        idx16 = np.tile(rows16, (8, 1))  # [128, 72]

        smalls_b = np.zeros((128, SMALL_COLS), np.float32)
        smalls_b[:, IDX0:IDX1] = idx16.view(np.float32)
        smalls_b[:, REM0:REM1] = _grid((flat % CHUNK).astype(np.float32))
        smalls_b[0:50, G0:G1] = gt[b][:, [0, 2]]
        smalls_b[50:100, G0:G1] = gt[b][:, [1, 3]]
        smalls_b[:, INVDA] = inv_da
        smalls_b[:, WF0:WF1] = _WF2D
        smalls_b[:, WS0:WS1] = _WS2D
        smalls_b[:, C0:C1] = _C2D
        smalls_b[:, IO0:IO1] = iota
        in_maps.append({"planes": planes_b, "smalls": smalls_b})
    return in_maps


def run(in_maps, trace=False):
    from concourse.bass_utils import run_bass_kernel_spmd

    if "nc" not in _CACHE:
        _CACHE["nc"] = build_bass()
    res = run_bass_kernel_spmd(
        _CACHE["nc"], in_maps, core_ids=list(range(N_CORES)), trace=trace
    )
    return res


def kernel(regression_targets, classification_targets, gt_boxes, loc, size,
           clf, occupancy, angle, heading, anchor):
    in_maps = host_inputs(regression_targets, classification_targets, gt_boxes,
                          loc, clf, anchor)
    res = run(in_maps)
    total = np.float32(0.0)
    for r in res.results:
        total += np.float32(r["out"][0, 0])
    return np.array(total, dtype=np.float32)



# revision 2
# speedup vs baseline: 1.2442x; 1.2442x over previous
"""PointPillar loss on 8 Trainium2 NeuronCores.

Data-parallel over the batch dim (B=8 -> one batch element per core).
Each core gathers the ~1150 elements of loc/clf that the loss actually
touches with a prepared+triggered dma_gather (256B rows), selects them
with 32-wide host-built one-hot masks (the planes are staged twice, the
second copy phase-shifted by 32 elements, so every value sits in the
first 32 lanes of its row), runs the focal / smooth-L1 math on DVE+ACT,
and scatter-adds the 128 per-partition partial sums into a
zero-initialized [128, 64] DRAM output via a second prepared descriptor.
The host sums the 8x128x2 partials.

Self-contained: hardcodes the problem shapes from the spec.
"""

import sys

import numpy as np

if "/opt/trn_rl_repo" not in sys.path:
    sys.path.insert(0, "/opt/trn_rl_repo")

B, A, H, W = 8, 2, 496, 432
N_BOXES, N_BG = 50, 1000
PLANE = H * W  # 214272
N_CORES = 8
COLS = 9
N_SLOTS = 128 * COLS  # 1152 slots; 1150 used
CHUNK = 64            # dma_gather row size in f32 elements (256B)
N_ROWS_HALF = 4 * PLANE // CHUNK  # 13392 rows per phase copy
N_ROWS = 2 * N_ROWS_HALF          # phase-0 copy then phase-32 copy
SELW = 32             # one-hot select width after the phase trick
ALPHA = 0.25
BETA_LOC = 2.0

# smalls[128, 52] column layout (f32 view)
IDX0, IDX1 = 0, 36     # gather row indices, int16 bits ([128, 72] i16)
SCI0, SCI1 = 36, 40    # scatter-add row indices, int16 ([128, 8] i16)
TG = 40                # per-partition gt target (x_gt / y_gt)
INVDA = 41             # 1/sqrt(anchor_w^2 + anchor_h^2)
WS0 = 42               # smooth-L1 weight (col-0 partitions 0..99)
SW0, SW1 = 43, 52      # sqrt(-focal weight) per slot, 0 on non-focal
SMALL_COLS = 52

_CACHE = {}


def _grid(flat):
    """Map a length-1152 slot vector to the on-chip [128, 9] layout.

    Slot n lives at partition n % 128, free column n // 128 (dma_gather's
    native output order) — so slots 0..99 (the smooth-L1 entries) occupy
    column 0, one per partition, letting the gt target act as a
    per-partition scalar operand.
    """
    return np.ascontiguousarray(flat.reshape(COLS, 128).T)


def build_bass():
    import concourse.bacc as bacc
    import concourse.bass as bass
    import concourse.mybir as mybir
    from concourse.library_config import mlp
    from contextlib import ExitStack

    f32 = mybir.dt.float32
    i16 = mybir.dt.int16
    op = mybir.AluOpType
    act = mybir.ActivationFunctionType

    nc = bacc.Bacc("TRN2", target_bir_lowering=False, debug=False,
                   num_devices=N_CORES)
    planes = nc.dram_tensor("planes", [N_ROWS, CHUNK], f32, kind="ExternalInput")
    smalls = nc.dram_tensor("smalls", [128, SMALL_COLS], f32,
                            kind="ExternalInput")
    maskd = nc.dram_tensor("maskd", [128, COLS * SELW], f32,
                           kind="ExternalInput")
    outp = nc.dram_tensor("out", [128, CHUNK], f32, kind="ExternalOutput")

    with ExitStack() as ctx:
        block = ctx.enter_context(nc.Block())

        def sb(name, shape, dt=f32):
            return ctx.enter_context(nc.sbuf_tensor(name, shape, dt))

        sm = sb("sm", [128, SMALL_COLS])
        msk = sb("msk", [128, COLS, SELW])
        v64 = sb("v64", [128, COLS, CHUNK])
        vm = sb("vm", [128, COLS, SELW])
        v = sb("v", [128, COLS])
        pcl = sb("pcl", [128, COLS])
        c1 = sb("c1", [128, COLS])
        cbw = sb("cbw", [128, COLS])
        c2bw = sb("c2bw", [128, COLS])
        lnb = sb("lnb", [128, COLS])
        junk9 = sb("junk9", [128, COLS])
        t = sb("t", [128, 1])
        ab = sb("ab", [128, 1])
        q = sb("q", [128, 1])
        mm1 = sb("mm1", [128, 1])
        r = sb("r", [128, 1])
        s = sb("s", [128, 1])
        big64 = sb("big64", [128, 1, CHUNK])
        warm = sb("warm", [1, 1])

        io = ctx.enter_context(nc.semaphore("io"))
        io2 = ctx.enter_context(nc.semaphore("io2"))
        iom = ctx.enter_context(nc.semaphore("iom"))
        bz = ctx.enter_context(nc.semaphore("bz"))
        zdone = ctx.enter_context(nc.semaphore("zdone"))
        gs = ctx.enter_context(nc.semaphore("gs"))
        od = ctx.enter_context(nc.semaphore("od"))
        p0s = ctx.enter_context(nc.semaphore("p0s"))
        p1s = ctx.enter_context(nc.semaphore("p1s"))
        dve_p = ctx.enter_context(nc.semaphore("dve_p"))
        act_done = ctx.enter_context(nc.semaphore("act_done"))
        dve_done = ctx.enter_context(nc.semaphore("dve_done"))
        dve_c = ctx.enter_context(nc.semaphore("dve_c"))

        ks = {}

        @block.vector
        def _(d: bass.BassVectorEngine):
            # Every DVE op incs dve_c at completion; dependent ops wait for
            # their producers' counts. Same-engine program order alone does
            # NOT make writes visible on this HW.
            cnt = [0]

            def step(ins):
                ins.then_inc(dve_c, 1)
                cnt[0] += 1
                return cnt[0]

            def need(k):
                d.wait_ge(dve_c, k)

            d.memset(big64[:], 0.0).then_inc(bz, 1)
            d.wait_ge(iom, 16)
            d.wait_ge(io2, 16)
            d.wait_ge(gs, 16)
            # select: vm = v64[:, :, 0:32] * one-hot, then row-sum per column
            mult_k = step(d.tensor_tensor(
                out=vm[:], in0=v64[:, :, 0:SELW], in1=msk[:], op=op.mult
            ))
            need(mult_k)
            red_k = step(d.tensor_reduce(
                out=v[:], in_=vm[:], axis=mybir.AxisListType.X, op=op.add
            ))
            need(red_k)
            # ln input first so ACT starts ASAP (inc goes to dve_p, not dve_c)
            pcl_ins = d.tensor_scalar(
                out=pcl[:], in0=v[:], scalar1=1e-12, scalar2=None, op0=op.max
            )
            pcl_ins.then_inc(dve_p, 1)
            ks["v"] = red_k
            # focal prep: c2bw = ((1-p) * sqrt(-wf))^2
            c1_k = step(d.tensor_scalar(
                out=c1[:], in0=v[:], scalar1=-1.0, scalar2=1.0,
                op0=op.mult, op1=op.add,
            ))
            need(c1_k)
            cbw_k = step(d.tensor_tensor(
                out=cbw[:], in0=c1[:], in1=sm[:, SW0:SW1], op=op.mult
            ))
            need(cbw_k)
            c2bw_k = step(d.tensor_tensor(
                out=c2bw[:], in0=cbw[:], in1=cbw[:], op=op.mult
            ))
            # smooth-L1 on column 0 only (x preds p0..49, y preds p50..99)
            t_k = step(d.tensor_scalar(
                out=t[:], in0=v[:, 0:1], scalar1=sm[:, TG:TG + 1],
                scalar2=sm[:, INVDA:INVDA + 1], op0=op.subtract, op1=op.mult,
            ))
            need(t_k)
            ab_k = step(d.scalar_tensor_tensor(
                out=ab[:], in0=t[:], scalar=-1.0, in1=t[:],
                op0=op.mult, op1=op.max,
            ))
            step(d.tensor_tensor(out=q[:], in0=t[:], in1=t[:], op=op.mult))
            need(ab_k)
            mm1_k = step(d.tensor_scalar(
                out=mm1[:], in0=ab[:], scalar1=1.0, scalar2=-1.0,
                op0=op.max, op1=op.add,
            ))
            need(mm1_k)
            r_k = step(d.tensor_tensor(out=r[:], in0=mm1[:], in1=mm1[:],
                                       op=op.mult))
            need(r_k)  # q completed earlier; cumulative count covers it
            s_k = step(d.tensor_tensor(out=s[:], in0=q[:], in1=r[:],
                                       op=op.subtract))
            need(s_k)
            d.wait_ge(zdone, 16)  # out DRAM zero-fill has read big64
            step(d.tensor_scalar(
                out=big64[:, 0, 1:2], in0=s[:], scalar1=sm[:, WS0:WS0 + 1],
                scalar2=None, op0=op.mult,
            ))
            need(c2bw_k)
            d.wait_ge(act_done, 1)  # lnb visible
            d.scalar_tensor_tensor(
                out=junk9[:], in0=c2bw[:], scalar=-1.0, in1=lnb[:],
                op0=op.mult, op1=op.mult, accum_out=big64[:, 0, 0:1],
            ).then_inc(dve_done, 1)

        @block.gpsimd
        def _(g: bass.BassGpSimd):
            g.load_library(mlp)
            nreg = g.to_reg(N_SLOTS)
            nreg128 = g.to_reg(128)
            g.wait_ge(io, 16)
            # single_packet=False: 1152 idxs -> 73 descriptors per lane, far
            # beyond the 64-descriptor/16KB single-packet limit.
            g.dma_gather(
                v64[:], planes[:], sm[:, IDX0:IDX1].bitcast(i16),
                N_SLOTS, nreg, CHUNK, single_packet=False,
                prepare_only=True, sem=gs,
            ).then_inc(p0s, 1)
            g.wait_ge(p0s, 1)
            g.trigger_dma(1)
            g.wait_ge(io2, 16)
            g.dma_scatter_add(
                outp[:], big64[:], sm[:, SCI0:SCI1].bitcast(i16),
                128, nreg128, CHUNK, prepare_only=True, sem=od,
            ).then_inc(p1s, 1)
            g.wait_ge(p1s, 1)
            g.wait_ge(zdone, 16)
            g.wait_ge(dve_done, 1)
            g.trigger_dma(1)

        @block.sync
        def _(sync: bass.BassEngine):
            # idx columns first: the gather only needs these (io >= 16).
            sync.dma_start(out=sm[:, IDX0:IDX1], in_=smalls[:, IDX0:IDX1]
                           ).then_inc(io, 16)
            sync.dma_start(out=sm[:, IDX1:], in_=smalls[:, IDX1:]
                           ).then_inc(io2, 16)
            sync.dma_start(out=msk[:], in_=maskd[:]).then_inc(iom, 16)
            sync.wait_ge(bz, 1)
            # zero-fill the output accumulator in DRAM (big64 is still zero)
            sync.dma_start(out=outp[:], in_=big64[:]).then_inc(zdone, 16)
            sync.wait_ge(od, 16)

        @block.scalar
        def _(sc: bass.BassScalarEngine):
            # warm the Ln table immediately (const input, no DMA dep)
            sc.activation(warm[:], nc.const_aps.tensor(1.0, (1, 1)), act.Ln)
            sc.wait_ge(dve_p, 1)
            sc.activation(lnb[:], pcl[:], act.Ln).then_inc(act_done, 1)

    nc.compile()
    return nc


def host_inputs(regression_targets, classification_targets, gt_boxes, loc, clf,
                anchor):
    reg = np.asarray(regression_targets).astype(np.int64)
    cls_t = np.asarray(classification_targets).astype(np.int64)
    gt = np.asarray(gt_boxes, dtype=np.float32)
    loc = np.asarray(loc, dtype=np.float32)
    clf = np.asarray(clf, dtype=np.float32)
    anc = np.asarray(anchor, dtype=np.float32)
    inv_da = np.float32(1.0) / np.sqrt(anc[0] * anc[0] + anc[1] * anc[1],
                                       dtype=np.float32)

    # per-slot constants shared by all cores
    sw = np.zeros(N_SLOTS, np.float32)
    sw[100:150] = np.sqrt(ALPHA / ((B - 1) * (N_BOXES - 1)))
    sw[150:1150] = np.sqrt(ALPHA / ((B - 1) * (N_BG - 1)))
    sw2d = _grid(sw)
    ws0 = np.zeros(128, np.float32)
    ws0[0:100] = 0.5 * BETA_LOC / (B * N_BOXES)

    # scatter-add idx: token n -> out row n; idx n at partition n % 16,
    # column n // 16, replicated across the 8 groups of 16 partitions.
    sci = np.ascontiguousarray(
        np.arange(128, dtype=np.int16).reshape(8, 16).T
    )
    sci128 = np.tile(sci, (8, 1))  # [128, 8] i16

    in_maps = []
    for b in range(B):
        full = np.ascontiguousarray(
            np.stack([loc[b, 0, 0], loc[b, 0, 1], clf[b, 0, 1], clf[b, 0, 0]])
        ).reshape(-1)  # [4*PLANE]
        full_pad = np.concatenate([full, np.zeros(CHUNK, np.float32)])
        planes_b = np.concatenate([
            full[: 4 * PLANE].reshape(N_ROWS_HALF, CHUNK),
            full_pad[32: 4 * PLANE + 32].reshape(N_ROWS_HALF, CHUNK),
        ])  # [N_ROWS, CHUNK]

        y, x = reg[b, :, 1], reg[b, :, 0]
        base = y * W + x
        flat = np.zeros(N_SLOTS, np.int64)
        flat[0:50] = 0 * PLANE + base
        flat[50:100] = 1 * PLANE + base
        flat[100:150] = 2 * PLANE + base
        flat[150:1150] = 3 * PLANE + cls_t[b, :, 2] * W + cls_t[b, :, 1]

        rem64 = flat % CHUNK
        use_b = rem64 >= SELW
        rows = np.where(use_b, (flat - SELW) // CHUNK + N_ROWS_HALF,
                        flat // CHUNK)
        rem = (flat % SELW).astype(np.int64)

        # dma_gather index layout: index n sits at partition n % 16,
        # column n // 16, replicated across the 8 groups of 16 partitions.
        rows16 = np.ascontiguousarray(
            rows.astype(np.int16).reshape(N_SLOTS // 16, 16).T
        )
        idx16 = np.tile(rows16, (8, 1))  # [128, 72]

        # one-hot masks [128, 9, 32]; zero on pad slots
        rem2d = _grid(rem.astype(np.float32))  # [128, 9]
        onehot = (np.arange(SELW, dtype=np.float32)[None, None, :]
                  == rem2d[:, :, None]).astype(np.float32)
        onehot[126:128, 8, :] = 0.0  # pad slots 1150, 1151
        maskd_b = np.ascontiguousarray(onehot.reshape(128, COLS * SELW))

        smalls_b = np.zeros((128, SMALL_COLS), np.float32)
        smalls_b[:, IDX0:IDX1] = idx16.view(np.float32)
        smalls_b[:, SCI0:SCI1] = sci128.view(np.float32)
        x_gt = gt[b][:, 0] + (gt[b][:, 2] - gt[b][:, 0]) * 0.5
        y_gt = gt[b][:, 1] - (gt[b][:, 3] - gt[b][:, 1]) * 0.5
        smalls_b[0:50, TG] = x_gt
        smalls_b[50:100, TG] = y_gt
        smalls_b[:, INVDA] = inv_da
        smalls_b[:, WS0] = ws0
        smalls_b[:, SW0:SW1] = sw2d
        in_maps.append({"planes": planes_b, "smalls": smalls_b,
                        "maskd": maskd_b})
    return in_maps


def run(in_maps, trace=False):
    from concourse.bass_utils import run_bass_kernel_spmd

    if "nc" not in _CACHE:
        _CACHE["nc"] = build_bass()
    res = run_bass_kernel_spmd(
        _CACHE["nc"], in_maps, core_ids=list(range(N_CORES)), trace=trace
    )
    return res


def kernel(regression_targets, classification_targets, gt_boxes, loc, size,
           clf, occupancy, angle, heading, anchor):
    in_maps = host_inputs(regression_targets, classification_targets, gt_boxes,
                          loc, clf, anchor)
    res = run(in_maps)
    total = np.float32(0.0)
    for r in res.results:
        out = np.asarray(r["out"], dtype=np.float32)
        total += np.float32(out[:, 0].sum() + out[:, 1].sum())
    return np.array(total, dtype=np.float32)


# revision 28
# speedup vs baseline: 1.6303x; 1.3103x over previous
"""PointPillar loss on 8 Trainium2 NeuronCores.

Data-parallel over the batch dim (B=8 -> one batch element per core).
Each core gathers the ~1150 elements of loc/clf that the loss actually
touches with a prepared+triggered dma_gather of 64B descriptors (32 bf16
lanes read from 256B-stride rows; the bf16 planes are staged four times
at 32-element phase shifts so every value sits in the first 32 lanes of
its row), selects them with host-built one-hot masks, runs the focal /
smooth-L1 math on DVE+ACT (the loc planes carry a +8 offset so Ln input
stays positive), and scatter-adds the 128 per-partition partial pairs
into a zero-initialized DRAM output via a second prepared descriptor.
The host sums the 8x128x2 partials. The framework's entry barrier is
reordered post-compile so the const-pool memsets stop gating engine
start.

Self-contained: hardcodes the problem shapes from the spec.
"""

import sys

import numpy as np

try:
    from ml_dtypes import bfloat16 as bf16_dt
except ImportError:  # pragma: no cover
    import jax.numpy as jnp
    bf16_dt = jnp.bfloat16

if "/opt/trn_rl_repo" not in sys.path:
    sys.path.insert(0, "/opt/trn_rl_repo")

B, A, H, W = 8, 2, 496, 432
N_BOXES, N_BG = 50, 1000
PLANE = H * W  # 214272
N_CORES = 8
COLS = 9
N_SLOTS = 128 * COLS  # 1152 slots; 1150 used
CHUNK = 64            # scatter row size in f32 elements (256B)
GROW = 128            # dma_gather row size in bf16 elements (256B)
N_ROWS_PH = 4 * PLANE // GROW  # 6696 rows per phase copy
N_PHASES = 4                   # copies phase-shifted by 32 elements
N_ROWS = N_PHASES * N_ROWS_PH
SELW = 32             # one-hot select width after the phase trick
ALPHA = 0.25
BETA_LOC = 2.0
LOC_OFF = 8.0  # positivity offset for the loc planes (see host_inputs)

# smalls[128, 52] column layout (f32 view)
IDX0, IDX1 = 0, 36     # gather row indices, int16 bits ([128, 72] i16)
SCI0, SCI1 = 36, 40    # scatter-add row indices, int16 ([128, 8] i16)
TG = 40                # per-partition gt target (x_gt / y_gt)
INVDA = 41             # 1/sqrt(anchor_w^2 + anchor_h^2)
WS0 = 42               # smooth-L1 weight (col-0 partitions 0..99)
SW0, SW1 = 43, 52      # sqrt(-focal weight) per slot, 0 on non-focal
SMALL_COLS = 52

_CACHE = {}


def _grid(flat):
    """Map a length-1152 slot vector to the on-chip [128, 9] layout.

    Slot n lives at partition n % 128, free column n // 128 (dma_gather's
    native output order) — so slots 0..99 (the smooth-L1 entries) occupy
    column 0, one per partition, letting the gt target act as a
    per-partition scalar operand.
    """
    return np.ascontiguousarray(flat.reshape(COLS, 128).T)


def build_bass():
    import concourse.bacc as bacc
    import concourse.bass as bass
    import concourse.mybir as mybir
    from concourse.library_config import mlp
    from contextlib import ExitStack

    f32 = mybir.dt.float32
    i16 = mybir.dt.int16
    op = mybir.AluOpType
    act = mybir.ActivationFunctionType

    nc = bacc.Bacc("TRN2", target_bir_lowering=False, debug=False,
                   num_devices=N_CORES)
    bf16 = mybir.dt.bfloat16
    planes = nc.dram_tensor("planes", [N_ROWS, GROW], bf16,
                            kind="ExternalInput")
    smalls = nc.dram_tensor("smalls", [128, SMALL_COLS], f32,
                            kind="ExternalInput")
    maskd = nc.dram_tensor("maskd", [128, COLS * SELW], bf16,
                           kind="ExternalInput")
    outp = nc.dram_tensor("out", [128, CHUNK], f32, kind="ExternalOutput")

    with ExitStack() as ctx:
        block = ctx.enter_context(nc.Block())

        def sb(name, shape, dt=f32):
            return ctx.enter_context(nc.sbuf_tensor(name, shape, dt))

        sm = sb("sm", [128, SMALL_COLS])
        msk = sb("msk", [128, COLS, SELW], bf16)
        v64 = sb("v64", [128, COLS, SELW], bf16)
        vm = sb("vm", [128, COLS, SELW], bf16)
        v = sb("v", [128, COLS])
        pcl = sb("pcl", [128, COLS])
        c1 = sb("c1", [128, COLS])
        cbw = sb("cbw", [128, COLS])
        c2bw = sb("c2bw", [128, COLS])
        lnb = sb("lnb", [128, COLS])
        junk9 = sb("junk9", [128, COLS])
        t = sb("t", [128, 1])
        ab = sb("ab", [128, 1])
        q = sb("q", [128, 1])
        mm1 = sb("mm1", [128, 1])
        r = sb("r", [128, 1])
        s = sb("s", [128, 1])
        big64 = sb("big64", [128, 1, CHUNK])
        warm = sb("warm", [1, 1])

        io = ctx.enter_context(nc.semaphore("io"))
        zdone = ctx.enter_context(nc.semaphore("zdone"))
        gs = ctx.enter_context(nc.semaphore("gs"))
        od = ctx.enter_context(nc.semaphore("od"))
        prep = ctx.enter_context(nc.semaphore("prep"))
        act_done = ctx.enter_context(nc.semaphore("act_done"))
        dve_done = ctx.enter_context(nc.semaphore("dve_done"))
        dve_c = ctx.enter_context(nc.semaphore("dve_c"))

        ks = {}

        @block.vector
        def _(d: bass.BassVectorEngine):
            # Every DVE op incs dve_c at completion; dependent ops wait for
            # their producers' counts. Same-engine program order alone does
            # NOT make writes visible on this HW.
            cnt = [0]

            def step(ins):
                ins.then_inc(dve_c, 1)
                cnt[0] += 1
                return cnt[0]

            def need(k):
                d.wait_ge(dve_c, k)

            step(d.memset(big64[:], 0.0))
            d.wait_ge(io, 48)
            d.wait_ge(zdone, 16)  # out DRAM zero-fill has read big64
            d.wait_ge(gs, 16)
            # select: vm = v64[:, :, 0:32] * one-hot, then row-sum per column
            mult_k = step(d.tensor_tensor(
                out=vm[:], in0=v64[:], in1=msk[:], op=op.mult
            ))
            need(mult_k)
            red_k = step(d.tensor_reduce(
                out=v[:], in_=vm[:], axis=mybir.AxisListType.X, op=op.add
            ))
            need(red_k)
            ks["v"] = red_k
            # focal prep: cbw = (v-1) * (-sqrt(-wf)) = (1-p)sqrt(-wf);
            # smooth-L1 via huber2(t) = min(|t|,1) * (2|t| - min(|t|,1)).
            # Dependent pairs are spaced so wait releases overlap execution.
            cbw_k = step(d.scalar_tensor_tensor(
                out=cbw[:], in0=v[:], scalar=-1.0, in1=sm[:, SW0:SW1],
                op0=op.add, op1=op.mult,
            ))
            t_k = step(d.tensor_scalar(
                out=t[:], in0=v[:, 0:1], scalar1=sm[:, TG:TG + 1],
                scalar2=sm[:, INVDA:INVDA + 1], op0=op.subtract, op1=op.mult,
            ))
            need(t_k)  # also covers cbw
            ab_k = step(d.scalar_tensor_tensor(
                out=ab[:], in0=t[:], scalar=-1.0, in1=t[:],
                op0=op.mult, op1=op.max,
            ))
            c2bw_k = step(d.tensor_tensor(
                out=c2bw[:], in0=cbw[:], in1=cbw[:], op=op.mult
            ))
            need(ab_k)
            m_k = step(d.tensor_scalar(
                out=mm1[:], in0=ab[:], scalar1=1.0, scalar2=None, op0=op.min,
            ))
            need(m_k)
            u_k = step(d.scalar_tensor_tensor(
                out=q[:], in0=ab[:], scalar=2.0, in1=mm1[:],
                op0=op.mult, op1=op.subtract,
            ))
            need(u_k)
            # j9a = (m * ws) * u -> smooth-L1 partial
            ks["j9a"] = step(d.scalar_tensor_tensor(
                out=big64[:, 0, 1:2], in0=mm1[:], scalar=sm[:, WS0:WS0 + 1],
                in1=q[:], op0=op.mult, op1=op.mult,
            ))
            d.wait_ge(act_done, 1)  # lnb visible; c2bw covered by need(u_k)
            d.scalar_tensor_tensor(
                out=junk9[:], in0=c2bw[:], scalar=-1.0, in1=lnb[:],
                op0=op.mult, op1=op.mult, accum_out=big64[:, 0, 0:1],
            ).then_inc(dve_done, 1)

        @block.gpsimd
        def _(g: bass.BassGpSimd):
            g.load_library(mlp)
            nreg = g.to_reg(N_SLOTS)
            nreg128 = g.to_reg(128)
            g.wait_ge(io, 16)
            # Raw InstDMAGatherAnt: 64B descriptors (elem_size=32 bf16) on a
            # 256B row stride (elem_step=128). bass.dma_gather asserts
            # elem_size_bytes % 256 == 0, but only the stride is encoded in
            # 256B units; the phase copies put every needed value in the
            # first 32 lanes of its row, so 64B per descriptor suffices and
            # the transfer hits the per-descriptor floor.
            # single_packet=False: 1152 idxs -> 73 descriptors per lane, far
            # beyond the 64-descriptor/16KB single-packet limit.
            in_ap = planes[:][:, 0:SELW]
            _in = g.lower_ap_dma(in_ap, for_custom_bir_dma=True)
            gi = g.add_instruction(mybir.InstDMAGatherAnt(
                name=nc.get_next_instruction_name(),
                ins=[*_in, g.lower_ap(sm[:, IDX0:IDX1].bitcast(i16)),
                     g.lower_val_access(nreg)],
                outs=[g.lower_ap(v64[:])],
                transpose=False, num_idxs=N_SLOTS, elem_size=SELW,
                stride_bytes_256=GROW * 2 // 256, gen_mode=1,
                single_packet=False, queue_num=0, sbuf_tokens_per_rank=0,
                sbuf_free_dim_per_rank=0, sbuf_free_dim_pad_per_rank=0,
                sbuf_byte_offset=0,
            ))
            gi.then_inc(gs, 16)
            g._track_prepare_only(gi, 0).then_inc(prep, 1)
            g.wait_ge(prep, 1)
            g.trigger_dma(1)
            g.wait_ge(io, 32)
            g.dma_scatter_add(
                outp[:][:, 0:2], big64[:, :, 0:2], sm[:, SCI0:SCI1].bitcast(i16),
                128, nreg128, 2, elem_step=CHUNK, prepare_only=True, sem=od,
            ).then_inc(prep, 1)
            g.wait_ge(prep, 2)
            g.wait_ge(zdone, 16)
            g.wait_ge(dve_c, ks["j9a"])
            g.wait_ge(dve_done, 1)
            g.trigger_dma(1)

        @block.sync
        def _(sync: bass.BassEngine):
            # idx columns first: the gather only needs these (io >= 16).
            sync.dma_start(out=sm[:, IDX0:IDX1], in_=smalls[:, IDX0:IDX1]
                           ).then_inc(io, 16)
            sync.dma_start(out=sm[:, IDX1:], in_=smalls[:, IDX1:]
                           ).then_inc(io, 16)
            sync.dma_start(out=msk[:], in_=maskd[:]).then_inc(io, 16)
            sync.wait_ge(dve_c, 1)
            # zero-fill the output accumulator in DRAM (big64 is still zero)
            sync.dma_start(out=outp[:][:, 0:2], in_=big64[:, :, 0:2]
                           ).then_inc(zdone, 16)

        @block.scalar
        def _(sc: bass.BassScalarEngine):
            # warm the Ln table immediately (const input, no DMA dep);
            # Square co-resides in the natural_log set: no reloads
            sc.activation(warm[:], nc.const_aps.tensor(1.0, (1, 1)), act.Ln)
            sc.wait_ge(dve_c, ks["v"])
            # Ln reads v directly: prob slots are in (0,1); the x/y slots can
            # be <= 0 but the HW table returns a finite clamped value there
            # and the focal weight is 0, so the product contributes nothing.
            sc.activation(lnb[:], v[:], act.Ln).then_inc(act_done, 1)

    nc.compile()
    _hoist_entry_barrier(nc)
    return nc


def _hoist_entry_barrier(nc):
    """Reorder Pool's preamble so the entry barrier releases before the
    const-pool memsets.

    The Bass constructor emits [Memset x4 (const pool), Drain,
    EvSem(gather>=4), EvSem(release+=4)] on Pool ahead of the entry
    barrier, so every engine idles ~600ns while Pool writes constants
    nothing reads until microseconds later (the one early reader, the ACT
    Ln-table warm, only uses the result as a dummy table-load input).
    Moving Pool's barrier handshake first releases the other engines
    ~300ns earlier. Counts are preserved (the EvSems self-reset), so the
    identically-structured exit barrier is unaffected. If the preamble
    shape ever changes, leave it untouched.
    """
    bb = nc.m.functions[0].blocks[0]
    if bb.name != "main":
        return
    import concourse.mybir as mybir

    pool_idx = [i for i, ins in enumerate(bb.instructions)
                if getattr(ins, "engine", None) == mybir.EngineType.Pool
                and type(ins).__name__ != "InstUnconditionalBranch"]
    pool = [bb.instructions[i] for i in pool_idx]
    memsets = [x for x in pool if type(x).__name__ == "InstMemset"]
    drains = [x for x in pool if type(x).__name__ == "InstDrain"]
    evsems = [x for x in pool if type(x).__name__ == "InstEventSemaphore"]
    if (len(pool) != 7 or len(memsets) != 4 or len(drains) != 1
            or len(evsems) != 2):
        return
    if not all("barrier_" in str(e) for e in evsems):
        return
    reordered = evsems + drains + memsets
    for i, ins in zip(pool_idx, reordered):
        bb.instructions[i] = ins


def host_inputs(regression_targets, classification_targets, gt_boxes, loc, clf,
                anchor):
    reg = np.asarray(regression_targets).astype(np.int64)
    cls_t = np.asarray(classification_targets).astype(np.int64)
    gt = np.asarray(gt_boxes, dtype=np.float32)
    loc = np.asarray(loc, dtype=np.float32)
    clf = np.asarray(clf, dtype=np.float32)
    anc = np.asarray(anchor, dtype=np.float32)
    inv_da = np.float32(1.0) / np.sqrt(anc[0] * anc[0] + anc[1] * anc[1],
                                       dtype=np.float32)

    # per-slot constants shared by all cores
    sw = np.zeros(N_SLOTS, np.float32)
    sw[100:150] = np.sqrt(ALPHA / ((B - 1) * (N_BOXES - 1)))
    sw[150:1150] = np.sqrt(ALPHA / ((B - 1) * (N_BG - 1)))
    sw2d = _grid(sw)
    ws0 = np.zeros(128, np.float32)
    ws0[0:100] = 0.5 * BETA_LOC / (B * N_BOXES)

    # scatter-add idx: token n -> out row n; idx n at partition n % 16,
    # column n // 16, replicated across the 8 groups of 16 partitions.
    sci = np.ascontiguousarray(
        np.arange(128, dtype=np.int16).reshape(8, 16).T
    )
    sci128 = np.tile(sci, (8, 1))  # [128, 8] i16

    in_maps = []
    for b in range(B):
        # +8 keeps every Ln input positive: the smooth-L1 slots gather
        # loc values (offset cancels against the shifted gt target), the
        # other slots gather probabilities in (0,1). P(loc < -8) ~ 6e-16.
        full = np.ascontiguousarray(
            np.stack([loc[b, 0, 0] + LOC_OFF, loc[b, 0, 1] + LOC_OFF,
                      clf[b, 0, 1], clf[b, 0, 0]])
        ).reshape(-1).astype(bf16_dt)  # [4*PLANE]
        full_pad = np.concatenate(
            [full, np.zeros(3 * SELW, bf16_dt)])
        planes_b = np.concatenate([
            full_pad[32 * k: 32 * k + 4 * PLANE].reshape(N_ROWS_PH, GROW)
            for k in range(N_PHASES)
        ])  # [N_ROWS, GROW]

        y, x = reg[b, :, 1], reg[b, :, 0]
        base = y * W + x
        flat = np.zeros(N_SLOTS, np.int64)
        flat[0:50] = 0 * PLANE + base
        flat[50:100] = 1 * PLANE + base
        flat[100:150] = 2 * PLANE + base
        flat[150:1150] = 3 * PLANE + cls_t[b, :, 2] * W + cls_t[b, :, 1]

        ph = (flat % GROW) // SELW
        rows = (flat - SELW * ph) // GROW + ph * N_ROWS_PH
        rem = (flat % SELW).astype(np.int64)

        # dma_gather index layout: index n sits at partition n % 16,
        # column n // 16, replicated across the 8 groups of 16 partitions.
        rows16 = np.ascontiguousarray(
            rows.astype(np.int16).reshape(N_SLOTS // 16, 16).T
        )
        idx16 = np.tile(rows16, (8, 1))  # [128, 72]

        # one-hot masks [128, 9, 32]; zero on pad slots
        rem2d = _grid(rem.astype(np.float32))  # [128, 9]
        onehot = (np.arange(SELW, dtype=np.float32)[None, None, :]
                  == rem2d[:, :, None]).astype(bf16_dt)
        # pad slots 1150/1151 keep their lane-0 one-hot: they read
        # planes[0][0] (positive, offset loc plane) so Ln stays finite;
        # their focal/smooth-L1 weights are zero.
        maskd_b = np.ascontiguousarray(onehot.reshape(128, COLS * SELW))

        smalls_b = np.zeros((128, SMALL_COLS), np.float32)
        smalls_b[:, IDX0:IDX1] = idx16.view(np.float32)
        smalls_b[:, SCI0:SCI1] = sci128.view(np.float32)
        x_gt = gt[b][:, 0] + (gt[b][:, 2] - gt[b][:, 0]) * 0.5
        y_gt = gt[b][:, 1] - (gt[b][:, 3] - gt[b][:, 1]) * 0.5
        smalls_b[0:50, TG] = x_gt + LOC_OFF
        smalls_b[50:100, TG] = y_gt + LOC_OFF
        smalls_b[:, INVDA] = inv_da
        smalls_b[:, WS0] = ws0
        smalls_b[:, SW0:SW1] = -sw2d
        in_maps.append({"planes": planes_b, "smalls": smalls_b,
                        "maskd": maskd_b})
    return in_maps


def run(in_maps, trace=False):
    from concourse.bass_utils import run_bass_kernel_spmd

    if "nc" not in _CACHE:
        _CACHE["nc"] = build_bass()
    res = run_bass_kernel_spmd(
        _CACHE["nc"], in_maps, core_ids=list(range(N_CORES)), trace=trace
    )
    return res


def kernel(regression_targets, classification_targets, gt_boxes, loc, size,
           clf, occupancy, angle, heading, anchor):
    in_maps = host_inputs(regression_targets, classification_targets, gt_boxes,
                          loc, clf, anchor)
    res = run(in_maps)
    total = np.float32(0.0)
    for r in res.results:
        out = np.asarray(r["out"], dtype=np.float32)
        total += np.float32(out[:, 0].sum() + out[:, 1].sum())
    return np.array(total, dtype=np.float32)


# revision 29
# speedup vs baseline: 1.6887x; 1.0358x over previous
"""PointPillar loss on 8 Trainium2 NeuronCores.

Data-parallel over the batch dim (B=8 -> one batch element per core).
Each core gathers the ~1150 elements of loc/clf that the loss actually
touches with a prepared+triggered dma_gather of 64B descriptors (32 bf16
lanes read from 256B-stride rows; the bf16 planes are staged four times
at 32-element phase shifts so every value sits in the first 32 lanes of
its row), selects them with host-built one-hot masks, runs the focal /
smooth-L1 math on DVE+ACT (the loc planes carry a +8 offset so Ln input
stays positive), and scatter-adds the 128 per-partition partial pairs
into a zero-initialized DRAM output via a second prepared descriptor.
The host sums the 8x128x2 partials. The framework's entry barrier is
reordered post-compile so the const-pool memsets stop gating engine
start.

Self-contained: hardcodes the problem shapes from the spec.
"""

import sys

import numpy as np

try:
    from ml_dtypes import bfloat16 as bf16_dt
except ImportError:  # pragma: no cover
    import jax.numpy as jnp
    bf16_dt = jnp.bfloat16

if "/opt/trn_rl_repo" not in sys.path:
    sys.path.insert(0, "/opt/trn_rl_repo")

B, A, H, W = 8, 2, 496, 432
N_BOXES, N_BG = 50, 1000
PLANE = H * W  # 214272
N_CORES = 8
COLS = 9
N_SLOTS = 128 * COLS  # 1152 slots; 1150 used
CHUNK = 64            # scatter row size in f32 elements (256B)
GROW = 128            # dma_gather row size in bf16 elements (256B)
N_ROWS_PH = 4 * PLANE // GROW  # 6696 rows per phase copy
N_PHASES = 4                   # copies phase-shifted by 32 elements
N_ROWS = N_PHASES * N_ROWS_PH
SELW = 32             # one-hot select width after the phase trick
ALPHA = 0.25
BETA_LOC = 2.0
LOC_OFF = 8.0  # positivity offset for the loc planes (see host_inputs)

# smalls[128, 52] column layout (f32 view)
IDX0, IDX1 = 0, 36     # gather row indices, int16 bits ([128, 72] i16)
SCI0, SCI1 = 36, 40    # scatter-add row indices, int16 ([128, 8] i16)
TG = 40                # per-partition gt target (x_gt / y_gt)
INVDA = 41             # 1/sqrt(anchor_w^2 + anchor_h^2)
WS0 = 42               # smooth-L1 weight (col-0 partitions 0..99)
SW0, SW1 = 43, 52      # sqrt(-focal weight) per slot, 0 on non-focal
SMALL_COLS = 52

_CACHE = {}


def _grid(flat):
    """Map a length-1152 slot vector to the on-chip [128, 9] layout.

    Slot n lives at partition n % 128, free column n // 128 (dma_gather's
    native output order) — so slots 0..99 (the smooth-L1 entries) occupy
    column 0, one per partition, letting the gt target act as a
    per-partition scalar operand.
    """
    return np.ascontiguousarray(flat.reshape(COLS, 128).T)


def build_bass():
    import concourse.bacc as bacc
    import concourse.bass as bass
    import concourse.mybir as mybir
    from concourse.library_config import mlp
    from contextlib import ExitStack

    f32 = mybir.dt.float32
    i16 = mybir.dt.int16
    op = mybir.AluOpType
    act = mybir.ActivationFunctionType

    nc = bacc.Bacc("TRN2", target_bir_lowering=False, debug=False,
                   num_devices=N_CORES)
    bf16 = mybir.dt.bfloat16
    planes = nc.dram_tensor("planes", [N_ROWS, GROW], bf16,
                            kind="ExternalInput")
    smalls = nc.dram_tensor("smalls", [128, SMALL_COLS], f32,
                            kind="ExternalInput")
    maskd = nc.dram_tensor("maskd", [128, COLS * SELW], bf16,
                           kind="ExternalInput")
    outp = nc.dram_tensor("out", [128, CHUNK], f32, kind="ExternalOutput")

    with ExitStack() as ctx:
        block = ctx.enter_context(nc.Block())

        def sb(name, shape, dt=f32):
            return ctx.enter_context(nc.sbuf_tensor(name, shape, dt))

        sm = sb("sm", [128, SMALL_COLS])
        msk = sb("msk", [128, COLS, SELW], bf16)
        v64 = sb("v64", [128, COLS, SELW], bf16)
        vm = sb("vm", [128, COLS, SELW], bf16)
        v = sb("v", [128, COLS])
        pcl = sb("pcl", [128, COLS])
        c1 = sb("c1", [128, COLS])
        cbw = sb("cbw", [128, COLS])
        c2bw = sb("c2bw", [128, COLS])
        lnb = sb("lnb", [128, COLS])
        junk9 = sb("junk9", [128, COLS])
        t = sb("t", [128, 1])
        ab = sb("ab", [128, 1])
        q = sb("q", [128, 1])
        mm1 = sb("mm1", [128, 1])
        r = sb("r", [128, 1])
        s = sb("s", [128, 1])
        big64 = sb("big64", [128, 1, CHUNK])
        warm = sb("warm", [1, 1])

        io = ctx.enter_context(nc.semaphore("io"))
        zdone = ctx.enter_context(nc.semaphore("zdone"))
        gs = ctx.enter_context(nc.semaphore("gs"))
        od = ctx.enter_context(nc.semaphore("od"))
        prep = ctx.enter_context(nc.semaphore("prep"))
        act_done = ctx.enter_context(nc.semaphore("act_done"))
        dve_done = ctx.enter_context(nc.semaphore("dve_done"))
        dve_c = ctx.enter_context(nc.semaphore("dve_c"))

        ks = {}

        @block.vector
        def _(d: bass.BassVectorEngine):
            # Every DVE op incs dve_c at completion; dependent ops wait for
            # their producers' counts. Same-engine program order alone does
            # NOT make writes visible on this HW.
            cnt = [0]

            def step(ins):
                ins.then_inc(dve_c, 1)
                cnt[0] += 1
                return cnt[0]

            def need(k):
                d.wait_ge(dve_c, k)

            step(d.memset(big64[:], 0.0))
            d.wait_ge(io, 48)
            d.wait_ge(zdone, 16)  # out DRAM zero-fill has read big64
            d.wait_ge(gs, 16)
            # select: vm = v64[:, :, 0:32] * one-hot, then row-sum per column
            mult_k = step(d.tensor_tensor(
                out=vm[:], in0=v64[:], in1=msk[:], op=op.mult
            ))
            need(mult_k)
            red_k = step(d.tensor_reduce(
                out=v[:], in_=vm[:], axis=mybir.AxisListType.X, op=op.add
            ))
            need(red_k)
            ks["v"] = red_k
            # focal prep: cbw = (v-1) * (-sqrt(-wf)) = (1-p)sqrt(-wf);
            # smooth-L1 via huber2(t) = min(|t|,1) * (2|t| - min(|t|,1)).
            # Dependent pairs are spaced so wait releases overlap execution.
            cbw_k = step(d.scalar_tensor_tensor(
                out=cbw[:], in0=v[:], scalar=-1.0, in1=sm[:, SW0:SW1],
                op0=op.add, op1=op.mult,
            ))
            t_k = step(d.tensor_scalar(
                out=t[:], in0=v[:, 0:1], scalar1=sm[:, TG:TG + 1],
                scalar2=sm[:, INVDA:INVDA + 1], op0=op.subtract, op1=op.mult,
            ))
            need(t_k)  # also covers cbw
            ab_k = step(d.scalar_tensor_tensor(
                out=ab[:], in0=t[:], scalar=-1.0, in1=t[:],
                op0=op.mult, op1=op.max,
            ))
            c2bw_k = step(d.tensor_tensor(
                out=c2bw[:], in0=cbw[:], in1=cbw[:], op=op.mult
            ))
            need(ab_k)
            m_k = step(d.tensor_scalar(
                out=mm1[:], in0=ab[:], scalar1=1.0, scalar2=None, op0=op.min,
            ))
            need(m_k)
            u_k = step(d.scalar_tensor_tensor(
                out=q[:], in0=ab[:], scalar=2.0, in1=mm1[:],
                op0=op.mult, op1=op.subtract,
            ))
            need(u_k)
            # j9a = (m * ws) * u -> smooth-L1 partial
            ks["j9a"] = step(d.scalar_tensor_tensor(
                out=big64[:, 0, 1:2], in0=mm1[:], scalar=sm[:, WS0:WS0 + 1],
                in1=q[:], op0=op.mult, op1=op.mult,
            ))
            d.wait_ge(act_done, 1)  # lnb visible; c2bw covered by need(u_k)
            d.scalar_tensor_tensor(
                out=junk9[:], in0=c2bw[:], scalar=-1.0, in1=lnb[:],
                op0=op.mult, op1=op.mult, accum_out=big64[:, 0, 0:1],
            ).then_inc(dve_done, 1)

        @block.gpsimd
        def _(g: bass.BassGpSimd):
            g.load_library(mlp)
            nreg = g.to_reg(N_SLOTS)
            nreg128 = g.to_reg(128)
            g.wait_ge(io, 16)
            # Raw InstDMAGatherAnt: 64B descriptors (elem_size=32 bf16) on a
            # 256B row stride (elem_step=128). bass.dma_gather asserts
            # elem_size_bytes % 256 == 0, but only the stride is encoded in
            # 256B units; the phase copies put every needed value in the
            # first 32 lanes of its row, so 64B per descriptor suffices and
            # the transfer hits the per-descriptor floor.
            # single_packet=False: 1152 idxs -> 73 descriptors per lane, far
            # beyond the 64-descriptor/16KB single-packet limit.
            in_ap = planes[:][:, 0:SELW]
            _in = g.lower_ap_dma(in_ap, for_custom_bir_dma=True)
            gi = g.add_instruction(mybir.InstDMAGatherAnt(
                name=nc.get_next_instruction_name(),
                ins=[*_in, g.lower_ap(sm[:, IDX0:IDX1].bitcast(i16)),
                     g.lower_val_access(nreg)],
                outs=[g.lower_ap(v64[:])],
                transpose=False, num_idxs=N_SLOTS, elem_size=SELW,
                stride_bytes_256=GROW * 2 // 256, gen_mode=1,
                single_packet=False, queue_num=0, sbuf_tokens_per_rank=0,
                sbuf_free_dim_per_rank=0, sbuf_free_dim_pad_per_rank=0,
                sbuf_byte_offset=0,
            ))
            gi.then_inc(gs, 16)
            g._track_prepare_only(gi, 0).then_inc(prep, 1)
            g.wait_ge(prep, 1)
            g.trigger_dma(1)
            g.wait_ge(io, 32)
            g.dma_scatter_add(
                outp[:][:, 0:2], big64[:, :, 0:2], sm[:, SCI0:SCI1].bitcast(i16),
                128, nreg128, 2, elem_step=CHUNK, prepare_only=True, sem=od,
            ).then_inc(prep, 1)
            g.wait_ge(prep, 2)
            g.wait_ge(zdone, 16)
            g.wait_ge(dve_c, ks["j9a"])
            g.wait_ge(dve_done, 1)
            g.trigger_dma(1)

        @block.sync
        def _(sync: bass.BassEngine):
            # idx columns first: the gather only needs these (io >= 16).
            sync.dma_start(out=sm[:, IDX0:IDX1], in_=smalls[:, IDX0:IDX1]
                           ).then_inc(io, 16)
            sync.dma_start(out=sm[:, IDX1:], in_=smalls[:, IDX1:]
                           ).then_inc(io, 16)
            sync.dma_start(out=msk[:], in_=maskd[:]).then_inc(io, 16)
            sync.wait_ge(dve_c, 1)
            # zero-fill the output accumulator in DRAM (big64 is still zero)
            sync.dma_start(out=outp[:][:, 0:2], in_=big64[:, :, 0:2]
                           ).then_inc(zdone, 16)

        @block.scalar
        def _(sc: bass.BassScalarEngine):
            # warm the Ln table immediately (const input, no DMA dep);
            # Square co-resides in the natural_log set: no reloads
            sc.activation(warm[:], nc.const_aps.tensor(1.0, (1, 1)), act.Ln)
            sc.wait_ge(dve_c, ks["v"])
            # Ln reads v directly: prob slots are in (0,1); the x/y slots can
            # be <= 0 but the HW table returns a finite clamped value there
            # and the focal weight is 0, so the product contributes nothing.
            sc.activation(lnb[:], v[:], act.Ln).then_inc(act_done, 1)

    nc.compile()
    _hoist_entry_barrier(nc)
    _hoist_input_dmas(nc)
    return nc


def _hoist_input_dmas(nc):
    """Issue the three input DMAs before SP's entry-barrier wait.

    They read staged DRAM and write SBUF regions nothing reads until the
    gather semaphores fire, so they need no cross-engine ordering. SP's
    barrier contribution is its entry Drain (which increments the gather
    count); only its release-wait must stay last, so inserting the DMAs
    between the two starts the index load at t~25 instead of t~300. The
    zero-fill DMA stays in the body (it waits on a DVE semaphore).
    """
    import concourse.mybir as mybir

    fn = nc.m.functions[0]
    main = fn.blocks[0]
    body = next((b for b in fn.blocks if "_SP_" in b.name), None)
    if body is None:
        return
    moved = []
    for ins in body.instructions:
        if (type(ins).__name__ == "InstDMACopy"
                and not (ins.sync_info and ins.sync_info.on_wait)):
            moved.append(ins)
        else:
            break
    if len(moved) != 3:
        return
    sp = mybir.EngineType.SP
    drain_i = next((i for i, ins in enumerate(main.instructions)
                    if getattr(ins, "engine", None) == sp
                    and type(ins).__name__ == "InstDrain"), None)
    if drain_i is None:
        return
    body.instructions = body.instructions[len(moved):]
    main.instructions = (main.instructions[:drain_i + 1] + moved
                         + main.instructions[drain_i + 1:])


def _hoist_entry_barrier(nc):
    """Reorder Pool's preamble so the entry barrier releases before the
    const-pool memsets.

    The Bass constructor emits [Memset x4 (const pool), Drain,
    EvSem(gather>=4), EvSem(release+=4)] on Pool ahead of the entry
    barrier, so every engine idles ~600ns while Pool writes constants
    nothing reads until microseconds later (the one early reader, the ACT
    Ln-table warm, only uses the result as a dummy table-load input).
    Moving Pool's barrier handshake first releases the other engines
    ~300ns earlier. Counts are preserved (the EvSems self-reset), so the
    identically-structured exit barrier is unaffected. If the preamble
    shape ever changes, leave it untouched.
    """
    bb = nc.m.functions[0].blocks[0]
    if bb.name != "main":
        return
    import concourse.mybir as mybir

    pool_idx = [i for i, ins in enumerate(bb.instructions)
                if getattr(ins, "engine", None) == mybir.EngineType.Pool
                and type(ins).__name__ != "InstUnconditionalBranch"]
    pool = [bb.instructions[i] for i in pool_idx]
    memsets = [x for x in pool if type(x).__name__ == "InstMemset"]
    drains = [x for x in pool if type(x).__name__ == "InstDrain"]
    evsems = [x for x in pool if type(x).__name__ == "InstEventSemaphore"]
    if (len(pool) != 7 or len(memsets) != 4 or len(drains) != 1
            or len(evsems) != 2):
        return
    if not all("barrier_" in str(e) for e in evsems):
        return
    reordered = evsems + drains + memsets
    for i, ins in zip(pool_idx, reordered):
        bb.instructions[i] = ins


def host_inputs(regression_targets, classification_targets, gt_boxes, loc, clf,
                anchor):
    reg = np.asarray(regression_targets).astype(np.int64)
    cls_t = np.asarray(classification_targets).astype(np.int64)
    gt = np.asarray(gt_boxes, dtype=np.float32)
    loc = np.asarray(loc, dtype=np.float32)
    clf = np.asarray(clf, dtype=np.float32)
    anc = np.asarray(anchor, dtype=np.float32)
    inv_da = np.float32(1.0) / np.sqrt(anc[0] * anc[0] + anc[1] * anc[1],
                                       dtype=np.float32)

    # per-slot constants shared by all cores
    sw = np.zeros(N_SLOTS, np.float32)
    sw[100:150] = np.sqrt(ALPHA / ((B - 1) * (N_BOXES - 1)))
    sw[150:1150] = np.sqrt(ALPHA / ((B - 1) * (N_BG - 1)))
    sw2d = _grid(sw)
    ws0 = np.zeros(128, np.float32)
    ws0[0:100] = 0.5 * BETA_LOC / (B * N_BOXES)

    # scatter-add idx: token n -> out row n; idx n at partition n % 16,
    # column n // 16, replicated across the 8 groups of 16 partitions.
    sci = np.ascontiguousarray(
        np.arange(128, dtype=np.int16).reshape(8, 16).T
    )
    sci128 = np.tile(sci, (8, 1))  # [128, 8] i16

    in_maps = []
    for b in range(B):
        # +8 keeps every Ln input positive: the smooth-L1 slots gather
        # loc values (offset cancels against the shifted gt target), the
        # other slots gather probabilities in (0,1). P(loc < -8) ~ 6e-16.
        full = np.ascontiguousarray(
            np.stack([loc[b, 0, 0] + LOC_OFF, loc[b, 0, 1] + LOC_OFF,
                      clf[b, 0, 1], clf[b, 0, 0]])
        ).reshape(-1).astype(bf16_dt)  # [4*PLANE]
        full_pad = np.concatenate(
            [full, np.zeros(3 * SELW, bf16_dt)])
        planes_b = np.concatenate([
            full_pad[32 * k: 32 * k + 4 * PLANE].reshape(N_ROWS_PH, GROW)
            for k in range(N_PHASES)
        ])  # [N_ROWS, GROW]

        y, x = reg[b, :, 1], reg[b, :, 0]
        base = y * W + x
        flat = np.zeros(N_SLOTS, np.int64)
        flat[0:50] = 0 * PLANE + base
        flat[50:100] = 1 * PLANE + base
        flat[100:150] = 2 * PLANE + base
        flat[150:1150] = 3 * PLANE + cls_t[b, :, 2] * W + cls_t[b, :, 1]

        ph = (flat % GROW) // SELW
        rows = (flat - SELW * ph) // GROW + ph * N_ROWS_PH
        rem = (flat % SELW).astype(np.int64)

        # dma_gather index layout: index n sits at partition n % 16,
        # column n // 16, replicated across the 8 groups of 16 partitions.
        rows16 = np.ascontiguousarray(
            rows.astype(np.int16).reshape(N_SLOTS // 16, 16).T
        )
        idx16 = np.tile(rows16, (8, 1))  # [128, 72]

        # one-hot masks [128, 9, 32]; zero on pad slots
        rem2d = _grid(rem.astype(np.float32))  # [128, 9]
        onehot = (np.arange(SELW, dtype=np.float32)[None, None, :]
                  == rem2d[:, :, None]).astype(bf16_dt)
        # pad slots 1150/1151 keep their lane-0 one-hot: they read
        # planes[0][0] (positive, offset loc plane) so Ln stays finite;
        # their focal/smooth-L1 weights are zero.
        maskd_b = np.ascontiguousarray(onehot.reshape(128, COLS * SELW))

        smalls_b = np.zeros((128, SMALL_COLS), np.float32)
        smalls_b[:, IDX0:IDX1] = idx16.view(np.float32)
        smalls_b[:, SCI0:SCI1] = sci128.view(np.float32)
        x_gt = gt[b][:, 0] + (gt[b][:, 2] - gt[b][:, 0]) * 0.5
        y_gt = gt[b][:, 1] - (gt[b][:, 3] - gt[b][:, 1]) * 0.5
        smalls_b[0:50, TG] = x_gt + LOC_OFF
        smalls_b[50:100, TG] = y_gt + LOC_OFF
        smalls_b[:, INVDA] = inv_da
        smalls_b[:, WS0] = ws0
        smalls_b[:, SW0:SW1] = -sw2d
        in_maps.append({"planes": planes_b, "smalls": smalls_b,
                        "maskd": maskd_b})
    return in_maps


def run(in_maps, trace=False):
    from concourse.bass_utils import run_bass_kernel_spmd

    if "nc" not in _CACHE:
        _CACHE["nc"] = build_bass()
    res = run_bass_kernel_spmd(
        _CACHE["nc"], in_maps, core_ids=list(range(N_CORES)), trace=trace
    )
    return res


def kernel(regression_targets, classification_targets, gt_boxes, loc, size,
           clf, occupancy, angle, heading, anchor):
    in_maps = host_inputs(regression_targets, classification_targets, gt_boxes,
                          loc, clf, anchor)
    res = run(in_maps)
    total = np.float32(0.0)
    for r in res.results:
        out = np.asarray(r["out"], dtype=np.float32)
        total += np.float32(out[:, 0].sum() + out[:, 1].sum())
    return np.array(total, dtype=np.float32)
